# revision 40
# baseline (speedup 1.0000x reference)
"""ContinuousMask kernel for Trainium2 (8 NeuronCores, SPMD row-sharded).

Problem: starts[B=2048, N=8192] int32, T=16384, l=1638. Output bool [B, T]:
True everywhere except the union of windows [s, s+l) over each row's starts.

Algorithm (per row):
  A position t is covered iff some start lies in (t-l, t]. With value-chunks
  of width W=512 (2W <= l), if every chunk 0..(smax>>9)-1 contains at least
  one start, then the covered region is EXACTLY [smin, smax+l):
    - t in [smin, smin+l): covered by the smin window.
    - t in [smin+l, smax): the previous chunk of t is nonempty; any start s'
      there satisfies t-l < s' <= t (since 2W <= l).
    - t in [smax, smax+l): covered by the smax window.
    - t < smin or t >= smax+l: no start in (t-l, t].
  The device computes smin, smax (full reduces) and an exact chunk-occupancy
  bitmask over a WITNESS SUBSET of columns (subset occupancy passing PROVES
  the condition; failing only flags the row for exact host recompute — on the
  target distribution a 2048-column witness fails with P ~ 1e-26). The
  occupancy requirement is strengthened to chunks 0..25 so that a passing row
  also has smin < 512 and smax >= 12800, which bounds the True runs to the
  painted head/tail strips. Flagged rows are recomputed exactly on host.

  The constant-zero middle of the mask is never stored: run_bass_kernel_spmd
  (both native and PJRT/axon paths) guarantees ExternalOutput buffers are
  zero-initialized (pre-zeroed / donated zero buffers), so only the head and
  tail strips are written.
"""

import numpy as np

B = 2048
T = 16384
NSEG = 8192
L = 1638
NCORES = 8
RPC = B // NCORES  # 256 rows per core
PT = 128  # rows per partition tile
NRT = RPC // PT  # 2 row tiles per core
SHIFT = 9  # occupancy chunk width 512 (2*512 <= L)
OCC_COLS = 2048  # occupancy witness column count (first chunk)
# Require witness occupancy of ALL chunks 0..28 (values span [0, 14747), so
# chunk 28 is the last). Chunk 0 occupied => smin < 512; chunk 28 occupied =>
# smax >= 14336 => the tail True-run starts at smax+L >= 15974. Chunk 28's
# witness expectation is ~57 hits (P(flag) ~ e^-57 per row).
MIN_CLAST = 29
HSTRIP = 512  # head strip [0, 512) covers [0, smin) since smin < 512
TSTART = T - 512  # tail strip [15872, T) covers runs starting >= 15974

_prog_cache: dict = {}


def _build_program(reps: int = 1, mode: str = "full"):
    """mode: 'full' | 'dma' (loads+stores only) | 'compute' (load once, compute reps x)."""
    import concourse.bacc as bacc
    import concourse.mybir as mybir
    from concourse.tile import TileContext

    dt = mybir.dt
    Alu = mybir.AluOpType
    X = mybir.AxisListType.X

    nc = bacc.Bacc("TRN2", debug=False)
    starts_d = nc.declare_dram_parameter("starts", [RPC, NSEG], dt.int32, isOutput=False)
    mask_d = nc.declare_dram_parameter("mask", [RPC, T], dt.uint8, isOutput=True)
    flags_d = nc.declare_dram_parameter("flags", [RPC, 1], dt.int32, isOutput=True)

    HALF = NSEG // 2
    with TileContext(nc) as tc:
        with (
            tc.tile_pool(name="persist", bufs=1) as pp,
            tc.tile_pool(name="stp", bufs=2) as stp,
            tc.tile_pool(name="strip", bufs=4) as outp,
            tc.tile_pool(name="work", bufs=1) as wp,
            tc.tile_pool(name="small", bufs=4) as sp,
        ):
            iota_t = pp.tile([PT, HSTRIP], dt.int16, tag="iota")
            nc.gpsimd.iota(iota_t[:], [[1, HSTRIP]], base=0, channel_multiplier=0)
            ones_t = pp.tile([PT, OCC_COLS], dt.int32, tag="ones")
            nc.vector.memset(ones_t[:], 1)

            persist_st: dict = {}
            for rep in range(reps):
              for rt in range(NRT):
                r0 = rt * PT
                do_load = mode != "compute" or rep == 0
                do_compute = mode != "dma"
                do_store = mode != "compute"

                if mode == "compute":
                    if rt not in persist_st:
                        st_persist = pp.tile([PT, NSEG], dt.int32, tag=f"st{rt}")
                        persist_st[rt] = st_persist
                    st = persist_st[rt]
                else:
                    st = stp.tile([PT, NSEG], dt.int32, tag="st")
                if do_load:
                    # two half-loads so reduces can start at half-load
                    nc.sync.dma_start(out=st[:, 0:HALF], in_=starts_d[r0 : r0 + PT, 0:HALF])
                    nc.sync.dma_start(out=st[:, HALF:NSEG], in_=starts_d[r0 : r0 + PT, HALF:NSEG])
                if not do_compute:
                    if do_store:
                        ph0 = outp.tile([PT, HSTRIP], dt.uint8, tag="ph")
                        nc.vector.memset(ph0[:], 0)
                        nc.scalar.dma_start(out=mask_d[r0 : r0 + PT, 0:HSTRIP], in_=ph0[:])
                        pt0 = outp.tile([PT, T - TSTART], dt.uint8, tag="pt")
                        nc.vector.memset(pt0[:], 0)
                        nc.scalar.dma_start(out=mask_d[r0 : r0 + PT, TSTART:T], in_=pt0[:])
                    continue

                # exact per-row min/max: partial reduce per half-load, combine
                smin = sp.tile([PT, 1], dt.int32, tag="smin")
                smax = sp.tile([PT, 1], dt.int32, tag="smax")
                mn1 = sp.tile([PT, 1], dt.int32, tag="mn1")
                mx1 = sp.tile([PT, 1], dt.int32, tag="mx1")
                nc.vector.tensor_reduce(smin[:], st[:, 0:HALF], X, Alu.min)
                nc.vector.tensor_reduce(smax[:], st[:, 0:HALF], X, Alu.max)
                nc.vector.tensor_reduce(mn1[:], st[:, HALF:NSEG], X, Alu.min)
                nc.vector.tensor_reduce(mx1[:], st[:, HALF:NSEG], X, Alu.max)
                nc.vector.tensor_tensor(smin[:], smin[:], mn1[:], Alu.min)
                nc.vector.tensor_tensor(smax[:], smax[:], mx1[:], Alu.max)

                # witness occupancy bitmask over the first OCC_COLS columns
                hi = wp.tile([PT, OCC_COLS], dt.int32, tag="hi")
                nc.vector.tensor_scalar(hi[:], st[:, 0:OCC_COLS], SHIFT, None, Alu.arith_shift_right)
                bits = wp.tile([PT, OCC_COLS], dt.int32, tag="bits")
                nc.vector.tensor_tensor(bits[:], ones_t[:], hi[:], Alu.logical_shift_left)
                w = OCC_COLS
                while w > 1:
                    h = w // 2
                    nc.vector.tensor_tensor(
                        bits[:, 0:h], bits[:, 0:h], bits[:, h:w], Alu.bitwise_or
                    )
                    w = h

                # flag = (occ | (-1 << MIN_CLAST)) != -1. Since MIN_CLAST=29
                # exceeds any clast (smax>>9 <= 28), max(clast, MIN_CLAST) is the
                # constant 29, so the mask is compile-time: -1<<29 = -2^29 (fp32-
                # exact immediate). Pure bitwise + fp32-safe compare.
                bad = sp.tile([PT, 1], dt.int32, tag="bad")
                nc.vector.tensor_scalar(bad[:], bits[:, 0:1], -(1 << MIN_CLAST), None, Alu.bitwise_or)
                nc.vector.tensor_scalar(bad[:], bad[:], -1.0, None, Alu.not_equal)
                if do_store:
                    nc.scalar.dma_start(out=flags_d[r0 : r0 + PT, :], in_=bad[:])

                # paint strips: head (t < smin) on DVE, tail (t >= smax+L-TSTART)
                # on GPSIMD; scalars prepared on ScalarE
                smin_f = sp.tile([PT, 1], dt.float32, tag="sminf")
                nc.scalar.copy(smin_f[:], smin[:])
                smaxl_f = sp.tile([PT, 1], dt.float32, tag="smaxlf")
                nc.scalar.activation(
                    smaxl_f[:], smax[:], mybir.ActivationFunctionType.Copy,
                    bias=float(L - TSTART), scale=1.0,
                )
                ph = outp.tile([PT, HSTRIP], dt.uint8, tag="ph")
                pt = outp.tile([PT, T - TSTART], dt.uint8, tag="pt")
                nc.vector.tensor_scalar(ph[:], iota_t[:], smin_f[:], None, Alu.is_lt)
                nc.gpsimd.tensor_scalar(pt[:], iota_t[:], smaxl_f[:], None, Alu.is_ge)
                if do_store:
                    nc.scalar.dma_start(out=mask_d[r0 : r0 + PT, 0:HSTRIP], in_=ph[:])
                    nc.scalar.dma_start(out=mask_d[r0 : r0 + PT, TSTART:T], in_=pt[:])

    nc.finalize()
    return nc


def _get_program(reps: int = 1, mode: str = "full"):
    key = (reps, mode)
    if key not in _prog_cache:
        _prog_cache[key] = _build_program(reps, mode)
    return _prog_cache[key]


def _host_exact_row(row_starts: np.ndarray) -> np.ndarray:
    delta = np.zeros(T + 1, np.int64)
    np.add.at(delta, row_starts, 1)
    np.add.at(delta, row_starts + L, -1)
    return ~(np.cumsum(delta)[:T] > 0)


def run_device(starts: np.ndarray, trace: bool = False):
    """Run the SPMD bass kernel. Returns (mask_u8 [B,T], flags [B], results)."""
    from concourse.bass_utils import run_bass_kernel_spmd

    nc = _get_program()
    shards = starts.reshape(NCORES, RPC, NSEG)
    in_maps = [{"starts": np.ascontiguousarray(shards[c])} for c in range(NCORES)]
    res = run_bass_kernel_spmd(nc, in_maps, list(range(NCORES)), trace=trace)
    mask = np.concatenate([r["mask"] for r in res.results], axis=0)
    flags = np.concatenate([r["flags"] for r in res.results], axis=0).reshape(-1)
    return mask, flags, res


def kernel(**inputs) -> np.ndarray:
    starts = np.ascontiguousarray(np.asarray(inputs["starts"]), dtype=np.int32)
    t_in = int(np.asarray(inputs["T"]))
    l_in = int(np.asarray(inputs["l"]))
    assert starts.shape == (B, NSEG), starts.shape
    assert t_in == T and l_in == L, (t_in, l_in)

    mask_u8, flags, _ = run_device(starts)
    mask = mask_u8.astype(bool)

    bad_rows = np.nonzero(flags != 0)[0]
    for r in bad_rows:  # pathological rows: exact host recompute (never on real data)
        mask[r] = _host_exact_row(starts[r])
    return mask
